# revision 26
# baseline (speedup 1.0000x reference)
"""Trainium2 Bass kernel for nn_Attention_86672440033867 (relative-position attention).

Sharding: head-parallel over 8 NeuronCores (1 head per core, all 16 batches).
Each core computes, for its head h:
  qkvT = w_qkv_h^T @ x^T           (M=96 chains -> qT/kT/vT rows)
  S^T  = k_b q_b^T                 (K=32 matmuls, keys on partitions)
  P^T  = exp(SCALE*S^T) * exp(B)^T (ACT exp + DVE/GPSIMD multiply; bias via
                                    host-gathered exp(bias) table, batch-invariant)
  flipped AV: per 128-query chunk, P^T chunks are the matmul STATIONARY
  ([128 keys, 128 q]) and v-natural rows (+ones column) are the moving tensor
  (33 rows instead of 128 per chunk-pair; LoadStationary is free on HW).
  The output lands token-major with the softmax denominator in column 32, so
  normalization is a per-partition reciprocal+scale; the normalized chunk is
  transposed back to O^T with one PE transpose per 512-token unit, and the
  projection reads the four 32-row bands via tile_position offsets.
  The dots matmuls (K=32) are spread across the four PE row-band quadrants
  (tile_position rows 0/32/64/96, with kT/qT replicated onto the matching
  partition bands), which the hardware executes concurrently.
Host sums the 8 partial projections and adds b_out.

Single merged software-pipelined loop over batches: while attention for batch
b runs, the qkv projection + v transposes for batch b+2 and the x DMA for
batch b+3 are interleaved into the same instruction stream, so the input DMA
and projection stage hide entirely under attention compute. All four engines
(PE/ACT/DVE/Pool) carry balanced elementwise load; PE is the binding engine
(the HW runs matmuls at the 1.2 GHz mid p-state).
"""
import numpy as np
import ml_dtypes
from contextlib import ExitStack, nullcontext

import concourse.bass as bass
import concourse.mybir as mybir
import concourse.tile as tile
from concourse import bacc
from concourse.bass_utils import run_bass_kernel_spmd

BF16 = mybir.dt.bfloat16
F32 = mybir.dt.float32

HEADS = 8
D = 32          # head dim
INP = 384
OUP = 384
SCALE = D ** -0.5
AF = mybir.ActivationFunctionType


def build_kernel(NB=16, N=1024, num_devices=8, loop_k=0):
    """Build the per-core Bass module. NB = total batches, N = tokens/batch."""
    assert NB % 4 == 0 and N % 128 == 0
    NJC = N // 128          # key chunks (128) per batch
    IH = min(512, N)        # query-column tile width
    NIH = N // IH           # query tiles per batch
    NTC = IH // 128         # token chunks (128) per query tile
    JG = min(2, NJC)        # j-chunks per exp/psum group
    NJG = (NJC + JG - 1) // JG
    TOK = NB * N

    nc = bacc.Bacc("TRN2", target_bir_lowering=False, num_devices=num_devices)

    xt_d = nc.dram_tensor("xt", [INP, TOK], BF16, kind="ExternalInput")
    wqkv_d = nc.dram_tensor("wqkv", [3, 128, 96], BF16, kind="ExternalInput")
    wout4_d = nc.dram_tensor("wout4", [128, OUP], BF16, kind="ExternalInput")
    expb_d = nc.dram_tensor("expb", [128, NJC, N], BF16, kind="ExternalInput")
    ident_d = nc.dram_tensor("ident", [128, 32], BF16, kind="ExternalInput")
    identb_d = nc.dram_tensor("identb", [128, 128], BF16, kind="ExternalInput")
    outp_d = nc.dram_tensor("outp", [TOK, OUP], BF16, kind="ExternalOutput")

    with tile.TileContext(nc) as tc, ExitStack() as ctx:
        const = ctx.enter_context(tc.tile_pool(name="const", bufs=1))
        big = ctx.enter_context(tc.tile_pool(name="big", bufs=1))

        wqkv_sb = const.tile([128, 3, 96], BF16)
        wout4_sb = const.tile([128, OUP], BF16)
        ident_sb = const.tile([128, 32], BF16)
        identb_sb = const.tile([128, 128], BF16)
        expb_sb = const.tile([128, NJC, N], BF16)
        for kc in range(3):
            nc.sync.dma_start(wqkv_sb[:, kc, :], wqkv_d.ap()[kc])
        nc.sync.dma_start(wout4_sb[:], wout4_d.ap())
        nc.sync.dma_start(ident_sb[:], ident_d.ap())
        nc.sync.dma_start(identb_sb[:], identb_d.ap())
        nc.sync.dma_start(expb_sb[:], expb_d.ap())

        # Resident activation layouts
        QKV = big.tile([96, TOK], BF16)              # rows: qT 0:32, kT 32:64, vT 64:96
        KB = big.tile([128, TOK], BF16)              # kT replicas on rows 0:32/64:96/96:128
        QB = big.tile([128, TOK], BF16)              # qT replicas on rows 32:64/64:96/96:128
        V_sb = big.tile([128, NB * NJC * 33], BF16)  # v natural [j,d] per (b,jc) + ones col
        OT = big.tile([128, TOK // 4], BF16)         # O^T blocks: rows 32c+d, unit-major

        nc.gpsimd.memset(V_sb[:], 1.0)  # ones column pre-fill; v blocks overwritten
        vv = V_sb[:].rearrange("p (b j e) -> p b j e", j=NJC, e=33)

        xt_pool = ctx.enter_context(tc.tile_pool(name="xt", bufs=4))
        es_pool = ctx.enter_context(tc.tile_pool(name="es", bufs=3))
        pt_pool = ctx.enter_context(tc.tile_pool(name="pt", bufs=2 * NJG + 2))
        out_pool = ctx.enter_context(tc.tile_pool(name="outp", bufs=3))
        rc_pool = ctx.enter_context(tc.tile_pool(name="rc", bufs=8))
        onat_pool = ctx.enter_context(tc.tile_pool(name="onat", bufs=4))

        xt3 = xt_d.ap().rearrange("(c p) (b t) -> p c b t", c=3, b=NB)  # [128,3,NB,N]
        outq = outp_d.ap().rearrange("(q p) o -> p q o", p=128)  # [128, TOK/128, OUP]

        ps_dots = ctx.enter_context(tc.tile_pool(name="ps_dots", bufs=2, space="PSUM"))
        ps_av = ctx.enter_context(tc.tile_pool(name="ps_av", bufs=2, space="PSUM"))
        ps_out = ctx.enter_context(tc.tile_pool(name="ps_out", bufs=1, space="PSUM"))
        ps_qv = ctx.enter_context(tc.tile_pool(name="ps_qv", bufs=1, space="PSUM"))

        def load_x(b):
            """One DMA: x^T tile for batch b."""
            t = xt_pool.tile([128, 3, N], BF16, tag="xt")
            nc.sync.dma_start(t[:], xt3[:, :, b, :])
            return t

        def replicate_kq(b):
            """Copy kT/qT to the PE row-band partitions for quadrant dots."""
            sl = slice(b * N, (b + 1) * N)
            nc.sync.dma_start(KB[0:32, sl], QKV[32:64, sl])
            nc.sync.dma_start(KB[64:96, sl], QKV[32:64, sl])
            nc.sync.dma_start(KB[96:128, sl], QKV[32:64, sl])
            nc.sync.dma_start(QB[32:64, sl], QKV[0:32, sl])
            nc.sync.dma_start(QB[64:96, sl], QKV[0:32, sl])
            nc.sync.dma_start(QB[96:128, sl], QKV[0:32, sl])

        def qkv_half(b, ih, xts):
            """qkv projection for tokens [b*N + ih*IH, +IH); evac on ACT."""
            i0 = b * N + ih * IH
            ps = ps_qv.tile([128, IH], F32, tag="ps_qv")
            for kc in range(3):
                nc.tensor.matmul(ps[0:96, :], wqkv_sb[:, kc, :],
                                 xts[:, kc, ih * IH:(ih + 1) * IH],
                                 start=(kc == 0), stop=(kc == 2))
            nc.scalar.activation(QKV[:, i0:i0 + IH], ps[0:96, :], AF.Copy)

        def vt_one(b, jc):
            """One v transpose: vT [32,128] -> v natural [128,32] for (b, jc)."""
            vt = ps_qv.tile([128, 1024], BF16, tag="ps_qv")
            nc.tensor.transpose(
                vt[:, 0:32],
                QKV[64:96, b * N + jc * 128:b * N + (jc + 1) * 128],
                ident_sb[64:96, 0:32],
                tile_position=(64, 0))
            nc.vector.tensor_copy(vv[:, b, jc, 0:32], vt[:, 0:32])

        def av_part(u, tcl):
            """Flipped AV for one 128-query chunk of unit u: P^T chunks are the
            stationary [128 keys, 128 q], v-natural rows (+ones col) move (33
            rows). Output lands token-major: avn [128 q, 33] with the softmax
            denominator in column 32; normalize per-partition (free)."""
            b, ih = u["b"], u["ih"]
            avn = ps_av.tile([128, 512], F32, tag="ps_av", name="avn")
            for jc in range(NJC):
                nc.tensor.matmul(
                    avn[:, 0:33],
                    u["pts"][jc // JG][:, (jc % JG) * IH + tcl * 128:
                                       (jc % JG) * IH + (tcl + 1) * 128],
                    vv[:, b, jc, 0:33],
                    start=(jc == 0), stop=(jc == NJC - 1))
            rc = rc_pool.tile([128, 1], F32, tag="rc")
            nc.vector.reciprocal(rc[:], avn[:, 32:33])
            if tcl == 0:
                u["onat"] = onat_pool.tile([128, NTC, 32], BF16, tag="onat",
                                           name="onat")
            nc.vector.tensor_scalar_mul(u["onat"][:, tcl, :], avn[:, 0:32], rc)

        def ot_part(u):
            """One merged transpose of all normalized chunks back to O^T."""
            b, ih = u["b"], u["ih"]
            i0 = b * N + ih * IH
            ott = ps_av.tile([128, 1024], BF16, tag="ps_av", name="ott")
            nc.tensor.transpose(ott[:, 0:128], u["onat"][:].rearrange("p a b -> p (a b)"),
                                identb_sb[:])
            nc.vector.tensor_copy(OT[:, i0 // 4:i0 // 4 + 128], ott[:, 0:128])

        def proj_part(u, tcl):
            """Output projection for one 128-token chunk of unit u."""
            b, ih = u["b"], u["ih"]
            i0 = b * N + ih * IH
            if tcl == 0:
                u["ot4"] = out_pool.tile([128, NTC, OUP], BF16, tag="outp", name="ot4")
            ot4 = u["ot4"]
            po = ps_out.tile([128, 512], F32, tag="ps_out")
            nc.tensor.matmul(
                po[:, 0:OUP],
                OT[32 * tcl:32 * tcl + 32, i0 // 4:i0 // 4 + 128],
                wout4_sb[32 * tcl:32 * tcl + 32, 0:OUP],
                start=True, stop=True, tile_position=(32 * tcl, 0))
            nc.vector.tensor_copy(ot4[:, tcl, :], po[:, 0:OUP])
            if tcl == NTC - 1:
                nc.sync.dma_start(outq[:, i0 // 128:i0 // 128 + NTC, :], ot4[:])

        # ---------------- prologue: x DMAs + qkv/vt for batches 0,1 -------------
        xts_ring = {}
        for b in range(3):
            xts_ring[b] = load_x(b)
        for b in range(2):
            for ih in range(NIH):
                qkv_half(b, ih, xts_ring[b])
            replicate_kq(b)
            for jc in range(NJC):
                vt_one(b, jc)

        # ---------------- merged steady-state loop over batches -----------------
        loopM = tc.For_i(0, loop_k, 1) if loop_k else nullcontext()
        with loopM:
            q = []  # software pipeline: av at U-1, proj at U-2
            mi = [0]

            def dots_grp(u, jg):
                b, ih, i0 = u["b"], u["ih"], u["b"] * N + u["ih"] * IH
                ps = ps_dots.tile([128, JG * IH], F32, tag="ps_dots")
                for r in range(JG):
                    jc = jg * JG + r
                    band = jc % 4
                    lo, hi = 32 * band, 32 * band + 32
                    kt = QKV[32:64, b * N + jc * 128:b * N + (jc + 1) * 128] \
                        if band == 1 else \
                        KB[lo:hi, b * N + jc * 128:b * N + (jc + 1) * 128]
                    qt = QKV[0:32, i0:i0 + IH] if band == 0 \
                        else QB[lo:hi, i0:i0 + IH]
                    nc.tensor.matmul(ps[:, r * IH:(r + 1) * IH], kt, qt,
                                     start=True, stop=True,
                                     tile_position=(lo, 0))
                es = es_pool.tile([128, JG * IH], BF16, tag="es")
                for r in range(JG):
                    nc.scalar.activation(es[:, r * IH:(r + 1) * IH],
                                         ps[:, r * IH:(r + 1) * IH],
                                         AF.Exp, scale=float(SCALE))
                pt = pt_pool.tile([128, JG * IH], BF16, tag="pt")
                for r in range(JG):
                    jc = jg * JG + r
                    eng = nc.vector if (mi[0] % 2 == 0) else nc.gpsimd
                    mi[0] += 1
                    eng.tensor_mul(
                        pt[:, r * IH:(r + 1) * IH],
                        es[:, r * IH:(r + 1) * IH],
                        expb_sb[:, jc, ih * IH:(ih + 1) * IH])
                u["pts"].append(pt)

            for b in range(NB):
                bp = (b + 2) % NB   # batch whose qkv/vt work rides along
                bd = (b + 3) % NB   # batch whose x DMA is issued
                xts_ring[bd] = load_x(bd)
                for ih in range(NIH):
                    u = {"b": b, "ih": ih, "pts": []}
                    u1 = q[-1] if len(q) >= 1 else None   # av target
                    u2 = q[-2] if len(q) >= 2 else None   # proj target
                    qkv_half(bp, ih, xts_ring[bp])
                    if ih == NIH - 1:
                        replicate_kq(bp)
                    dots_grp(u, 0)
                    if u1:
                        av_part(u1, 0)
                        av_part(u1, 1)
                    dots_grp(u, 1)
                    vt_one(bp, ih * NJG + 0)
                    vt_one(bp, ih * NJG + 1)
                    if u1:
                        av_part(u1, 2)
                    dots_grp(u, 2)
                    if u2:
                        proj_part(u2, 0)
                        proj_part(u2, 1)
                    if u1:
                        av_part(u1, 3)
                    dots_grp(u, 3)
                    vt_one(bp, ih * NJG + 2)
                    vt_one(bp, ih * NJG + 3)
                    if u1:
                        ot_part(u1)
                    if u2:
                        proj_part(u2, 2)
                        proj_part(u2, 3)
                    q.append(u)
            # drain the 2-unit pipeline tail
            for tcl in range(NTC):
                av_part(q[-1], tcl)
            ot_part(q[-1])
            for tcl in range(NTC):
                proj_part(q[-2], tcl)
            for tcl in range(NTC):
                proj_part(q[-1], tcl)
    nc.compile()
    return nc


def host_prep(x, w_qkv, relative_bias_table, relative_index, w_out, NB, N):
    """Build per-core input maps."""
    bf = ml_dtypes.bfloat16
    TOK = NB * N
    NJC = N // 128
    xt = np.ascontiguousarray(x.reshape(TOK, INP).T).astype(bf)
    ident = np.tile(np.eye(32, dtype=np.float32), (4, 1)).astype(bf)
    bias_full = relative_bias_table[relative_index]  # [N, N, H]
    in_maps = []
    for h in range(HEADS):
        w96 = np.concatenate(
            [w_qkv[:, h * D:(h + 1) * D],
             w_qkv[:, 256 + h * D:256 + (h + 1) * D],
             w_qkv[:, 512 + h * D:512 + (h + 1) * D]], axis=1)  # [384, 96]
        wqkv3 = np.ascontiguousarray(w96.reshape(3, 128, 96)).astype(bf)
        wout4 = np.tile(w_out[h * D:(h + 1) * D, :], (4, 1)).astype(np.float32)
        expbT = np.exp(bias_full[:, :, h].T)  # [j, i]
        expb = np.ascontiguousarray(
            expbT.reshape(NJC, 128, N).transpose(1, 0, 2)).astype(bf)
        in_maps.append({
            "xt": xt, "wqkv": wqkv3, "wout4": wout4.astype(bf),
            "expb": expb, "ident": ident,
            "identb": np.eye(128, dtype=np.float32).astype(bf),
        })
    return in_maps


_NC_CACHE = {}


def kernel(x, w_qkv, relative_bias_table, w_out, b_out, relative_index):
    x = np.asarray(x, dtype=np.float32)
    w_qkv = np.asarray(w_qkv, dtype=np.float32)
    relative_bias_table = np.asarray(relative_bias_table, dtype=np.float32)
    w_out = np.asarray(w_out, dtype=np.float32)
    b_out = np.asarray(b_out, dtype=np.float32)
    relative_index = np.asarray(relative_index)

    NB, N, _ = x.shape
    key = (NB, N)
    if key not in _NC_CACHE:
        _NC_CACHE[key] = build_kernel(NB=NB, N=N, num_devices=HEADS)
    nc = _NC_CACHE[key]

    in_maps = host_prep(x, w_qkv, relative_bias_table, relative_index, w_out, NB, N)
    res = run_bass_kernel_spmd(nc, in_maps, core_ids=list(range(HEADS)))
    out = np.zeros((NB * N, OUP), np.float32)
    for r in res.results:
        out += r["outp"].astype(np.float32)
    out += b_out[None, :]
    return out.reshape(NB, N, OUP)


# revision 27
# speedup vs baseline: 1.1049x; 1.1049x over previous
"""Trainium2 Bass kernel for nn_Attention_86672440033867 (relative-position attention).

Sharding: head-parallel over 8 NeuronCores (1 head per core, all 16 batches).
Each core computes, for its head h:
  qkvT = w_qkv_h^T @ x^T           (M=96 chains -> qT/kT/vT rows)
  S^T  = k_b q_b^T                 (K=32 matmuls, keys on partitions)
  P^T  = exp(SCALE*S^T) * exp(B)^T (ACT exp + DVE/GPSIMD multiply; bias via
                                    host-gathered exp(bias) table, batch-invariant)
  flipped AV: per 128-query chunk, P^T chunks are the matmul STATIONARY
  ([128 keys, 128 q]) and v-natural rows (+ones column) are the moving tensor
  (33 rows instead of 128 per chunk-pair; LoadStationary is free on HW).
  The output lands token-major with the softmax denominator in column 32, so
  normalization is a per-partition reciprocal+scale; the normalized chunk is
  transposed back to O^T with one PE transpose per 512-token unit, and the
  projection reads the four 32-row bands via tile_position offsets.
  The dots matmuls (K=32) are spread across the four PE row-band quadrants
  (tile_position rows 0/32/64/96, with kT/qT replicated onto the matching
  partition bands), which the hardware executes concurrently.
Host sums the 8 partial projections and adds b_out.

Single merged software-pipelined loop over batches: while attention for batch
b runs, the qkv projection + v transposes for batch b+2 and the x DMA for
batch b+3 are interleaved into the same instruction stream, so the input DMA
and projection stage hide entirely under attention compute. All four engines
(PE/ACT/DVE/Pool) carry balanced elementwise load; PE is the binding engine
(the HW runs matmuls at the 1.2 GHz mid p-state).
"""
import numpy as np
import ml_dtypes
from contextlib import ExitStack, nullcontext

import concourse.bass as bass
import concourse.mybir as mybir
import concourse.tile as tile
from concourse import bacc
from concourse.bass_utils import run_bass_kernel_spmd

BF16 = mybir.dt.bfloat16
F32 = mybir.dt.float32

HEADS = 8
D = 32          # head dim
INP = 384
OUP = 384
SCALE = D ** -0.5
AF = mybir.ActivationFunctionType


def build_kernel(NB=16, N=1024, num_devices=8, loop_k=0):
    """Build the per-core Bass module. NB = total batches, N = tokens/batch."""
    assert NB % 4 == 0 and N % 128 == 0
    NJC = N // 128          # key chunks (128) per batch
    IH = min(512, N)        # query-column tile width
    NIH = N // IH           # query tiles per batch
    NTC = IH // 128         # token chunks (128) per query tile
    JG = min(2, NJC)        # j-chunks per exp/psum group
    NJG = (NJC + JG - 1) // JG
    TOK = NB * N

    nc = bacc.Bacc("TRN2", target_bir_lowering=False, num_devices=num_devices)

    xt_d = nc.dram_tensor("xt", [INP, TOK], BF16, kind="ExternalInput")
    wqkv_d = nc.dram_tensor("wqkv", [3, 128, 96], BF16, kind="ExternalInput")
    wout4_d = nc.dram_tensor("wout4", [128, OUP], BF16, kind="ExternalInput")
    expb_d = nc.dram_tensor("expb", [128, NJC, N], BF16, kind="ExternalInput")
    ident_d = nc.dram_tensor("ident", [128, 32], BF16, kind="ExternalInput")
    identb_d = nc.dram_tensor("identb", [128, 128], BF16, kind="ExternalInput")
    outp_d = nc.dram_tensor("outp", [TOK, OUP], BF16, kind="ExternalOutput")

    with tile.TileContext(nc) as tc, ExitStack() as ctx:
        const = ctx.enter_context(tc.tile_pool(name="const", bufs=1))
        big = ctx.enter_context(tc.tile_pool(name="big", bufs=1))

        wqkv_sb = const.tile([128, 3, 96], BF16)
        wout4_sb = const.tile([128, OUP], BF16)
        ident_sb = const.tile([128, 32], BF16)
        identb_sb = const.tile([128, 128], BF16)
        expb_sb = const.tile([128, NJC, N], BF16)
        for kc in range(3):
            nc.sync.dma_start(wqkv_sb[:, kc, :], wqkv_d.ap()[kc])
        nc.sync.dma_start(wout4_sb[:], wout4_d.ap())
        nc.sync.dma_start(ident_sb[:], ident_d.ap())
        nc.sync.dma_start(identb_sb[:], identb_d.ap())
        nc.sync.dma_start(expb_sb[:], expb_d.ap())

        # Resident activation layouts
        QKV = big.tile([96, TOK], BF16)              # rows: qT 0:32, kT 32:64, vT 64:96
        KB = big.tile([128, TOK], BF16)              # kT replicas on rows 0:32/64:96/96:128
        QB = big.tile([128, TOK], BF16)              # qT replicas on rows 32:64/64:96/96:128
        V_sb = big.tile([128, NB * NJC * 33], BF16)  # v natural [j,d] per (b,jc) + ones col
        OT = big.tile([128, TOK // 4], BF16)         # O^T blocks: rows 32c+d, unit-major

        nc.gpsimd.memset(V_sb[:], 1.0)  # ones column pre-fill; v blocks overwritten
        vv = V_sb[:].rearrange("p (b j e) -> p b j e", j=NJC, e=33)

        xt_pool = ctx.enter_context(tc.tile_pool(name="xt", bufs=4))
        es_pool = ctx.enter_context(tc.tile_pool(name="es", bufs=3))
        pt_pool = ctx.enter_context(tc.tile_pool(name="pt", bufs=2 * NJG + 2))
        out_pool = ctx.enter_context(tc.tile_pool(name="outp", bufs=3))
        rc_pool = ctx.enter_context(tc.tile_pool(name="rc", bufs=8))
        onat_pool = ctx.enter_context(tc.tile_pool(name="onat", bufs=4))

        xt3 = xt_d.ap().rearrange("(c p) (b t) -> p c b t", c=3, b=NB)  # [128,3,NB,N]
        outq = outp_d.ap().rearrange("(q p) o -> p q o", p=128)  # [128, TOK/128, OUP]

        ps_dots = ctx.enter_context(tc.tile_pool(name="ps_dots", bufs=2, space="PSUM"))
        ps_av = ctx.enter_context(tc.tile_pool(name="ps_av", bufs=2, space="PSUM"))
        ps_out = ctx.enter_context(tc.tile_pool(name="ps_out", bufs=1, space="PSUM"))
        ps_qv = ctx.enter_context(tc.tile_pool(name="ps_qv", bufs=1, space="PSUM"))

        def load_x(b):
            """One DMA: x^T tile for batch b."""
            t = xt_pool.tile([128, 3, N], BF16, tag="xt")
            nc.sync.dma_start(t[:], xt3[:, :, b, :])
            return t

        def replicate_kq(b):
            """Copy kT/qT to the PE row-band partitions for quadrant dots."""
            sl = slice(b * N, (b + 1) * N)
            nc.sync.dma_start(KB[0:32, sl], QKV[32:64, sl])
            nc.sync.dma_start(KB[64:96, sl], QKV[32:64, sl])
            nc.sync.dma_start(KB[96:128, sl], QKV[32:64, sl])
            nc.sync.dma_start(QB[32:64, sl], QKV[0:32, sl])
            nc.sync.dma_start(QB[64:96, sl], QKV[0:32, sl])
            nc.sync.dma_start(QB[96:128, sl], QKV[0:32, sl])

        def qkv_half(b, ih, xts):
            """qkv projection for tokens [b*N + ih*IH, +IH); evac on ACT."""
            i0 = b * N + ih * IH
            ps = ps_qv.tile([128, IH], F32, tag="ps_qv")
            for kc in range(3):
                nc.tensor.matmul(ps[0:96, :], wqkv_sb[:, kc, :],
                                 xts[:, kc, ih * IH:(ih + 1) * IH],
                                 start=(kc == 0), stop=(kc == 2))
            nc.scalar.activation(QKV[:, i0:i0 + IH], ps[0:96, :], AF.Copy)

        def vt_one(b, jc):
            """One v transpose: vT [32,128] -> v natural [128,32] for (b, jc)."""
            vt = ps_qv.tile([128, 1024], BF16, tag="ps_qv")
            nc.tensor.transpose(
                vt[:, 0:32],
                QKV[64:96, b * N + jc * 128:b * N + (jc + 1) * 128],
                ident_sb[64:96, 0:32],
                tile_position=(64, 0))
            nc.vector.tensor_copy(vv[:, b, jc, 0:32], vt[:, 0:32])

        def av_part(u, tcl):
            """Flipped AV for one 128-query chunk of unit u: P^T chunks are the
            stationary [128 keys, 128 q], v-natural rows (+ones col) move (33
            rows). Output lands token-major: avn [128 q, 33] with the softmax
            denominator in column 32; normalize per-partition (free)."""
            b, ih = u["b"], u["ih"]
            avn = ps_av.tile([128, 512], F32, tag="ps_av", name="avn")
            for jc in range(NJC):
                nc.tensor.matmul(
                    avn[:, 0:33],
                    u["pts"][jc // JG][:, (jc % JG) * IH + tcl * 128:
                                       (jc % JG) * IH + (tcl + 1) * 128],
                    vv[:, b, jc, 0:33],
                    start=(jc == 0), stop=(jc == NJC - 1))
            rc = rc_pool.tile([128, 1], F32, tag="rc")
            nc.vector.reciprocal(rc[:], avn[:, 32:33])
            if tcl == 0:
                u["onat"] = onat_pool.tile([128, NTC, 32], BF16, tag="onat",
                                           name="onat")
            nc.vector.tensor_scalar_mul(u["onat"][:, tcl, :], avn[:, 0:32], rc)

        def ot_part(u):
            """One merged transpose of all normalized chunks back to O^T."""
            b, ih = u["b"], u["ih"]
            i0 = b * N + ih * IH
            ott = ps_av.tile([128, 1024], BF16, tag="ps_av", name="ott")
            nc.tensor.transpose(ott[:, 0:128], u["onat"][:].rearrange("p a b -> p (a b)"),
                                identb_sb[:])
            nc.vector.tensor_copy(OT[:, i0 // 4:i0 // 4 + 128], ott[:, 0:128])

        def proj_part(u, tcl):
            """Output projection for one 128-token chunk of unit u."""
            b, ih = u["b"], u["ih"]
            i0 = b * N + ih * IH
            if tcl == 0:
                u["ot4"] = out_pool.tile([128, NTC, OUP], BF16, tag="outp", name="ot4")
            ot4 = u["ot4"]
            po = ps_out.tile([128, 512], F32, tag="ps_out")
            nc.tensor.matmul(
                po[:, 0:OUP],
                OT[32 * tcl:32 * tcl + 32, i0 // 4:i0 // 4 + 128],
                wout4_sb[32 * tcl:32 * tcl + 32, 0:OUP],
                start=True, stop=True, tile_position=(32 * tcl, 0))
            nc.vector.tensor_copy(ot4[:, tcl, :], po[:, 0:OUP])
            if tcl == NTC - 1:
                nc.sync.dma_start(outq[:, i0 // 128:i0 // 128 + NTC, :], ot4[:])

        # ---------------- prologue: x DMAs + qkv/vt for batches 0,1 -------------
        xts_ring = {}
        for b in range(3):
            xts_ring[b] = load_x(b)
        for b in range(2):
            for ih in range(NIH):
                qkv_half(b, ih, xts_ring[b])
            replicate_kq(b)
            for jc in range(NJC):
                vt_one(b, jc)

        # ---------------- merged steady-state loop over batches -----------------
        loopM = tc.For_i(0, loop_k, 1) if loop_k else nullcontext()
        with loopM:
            q = []  # software pipeline: av at U-1, proj at U-2
            mi = [0]

            def dots_grp(u, jg):
                b, ih, i0 = u["b"], u["ih"], u["b"] * N + u["ih"] * IH
                ps = ps_dots.tile([128, JG * IH], F32, tag="ps_dots")
                for r in range(JG):
                    jc = jg * JG + r
                    band = jc % 4
                    lo, hi = 32 * band, 32 * band + 32
                    kt = QKV[32:64, b * N + jc * 128:b * N + (jc + 1) * 128] \
                        if band == 1 else \
                        KB[lo:hi, b * N + jc * 128:b * N + (jc + 1) * 128]
                    qt = QKV[0:32, i0:i0 + IH] if band == 0 \
                        else QB[lo:hi, i0:i0 + IH]
                    nc.tensor.matmul(ps[:, r * IH:(r + 1) * IH], kt, qt,
                                     start=True, stop=True,
                                     tile_position=(lo, 0))
                es = es_pool.tile([128, JG * IH], BF16, tag="es")
                nc.scalar.activation(es[:], ps[:], AF.Exp, scale=float(SCALE))
                pt = pt_pool.tile([128, JG * IH], BF16, tag="pt")
                for r in range(JG):
                    jc = jg * JG + r
                    eng = nc.vector if (mi[0] % 2 == 0) else nc.gpsimd
                    mi[0] += 1
                    eng.tensor_mul(
                        pt[:, r * IH:(r + 1) * IH],
                        es[:, r * IH:(r + 1) * IH],
                        expb_sb[:, jc, ih * IH:(ih + 1) * IH])
                u["pts"].append(pt)

            for b in range(NB):
                bp = (b + 2) % NB   # batch whose qkv/vt work rides along
                bd = (b + 3) % NB   # batch whose x DMA is issued
                xts_ring[bd] = load_x(bd)
                for ih in range(NIH):
                    u = {"b": b, "ih": ih, "pts": []}
                    u1 = q[-1] if len(q) >= 1 else None   # av target
                    u2 = q[-2] if len(q) >= 2 else None   # proj target
                    qkv_half(bp, ih, xts_ring[bp])
                    if ih == NIH - 1:
                        replicate_kq(bp)
                    dots_grp(u, 0)
                    if u1:
                        av_part(u1, 0)
                        av_part(u1, 1)
                    dots_grp(u, 1)
                    vt_one(bp, ih * NJG + 0)
                    vt_one(bp, ih * NJG + 1)
                    if u1:
                        av_part(u1, 2)
                    dots_grp(u, 2)
                    if u2:
                        proj_part(u2, 0)
                        proj_part(u2, 1)
                    if u1:
                        av_part(u1, 3)
                    dots_grp(u, 3)
                    vt_one(bp, ih * NJG + 2)
                    vt_one(bp, ih * NJG + 3)
                    if u1:
                        ot_part(u1)
                    if u2:
                        proj_part(u2, 2)
                        proj_part(u2, 3)
                    q.append(u)
            # drain the 2-unit pipeline tail
            for tcl in range(NTC):
                av_part(q[-1], tcl)
            ot_part(q[-1])
            for tcl in range(NTC):
                proj_part(q[-2], tcl)
            for tcl in range(NTC):
                proj_part(q[-1], tcl)
    nc.compile()
    return nc


def host_prep(x, w_qkv, relative_bias_table, relative_index, w_out, NB, N):
    """Build per-core input maps."""
    bf = ml_dtypes.bfloat16
    TOK = NB * N
    NJC = N // 128
    xt = np.ascontiguousarray(x.reshape(TOK, INP).T).astype(bf)
    ident = np.tile(np.eye(32, dtype=np.float32), (4, 1)).astype(bf)
    bias_full = relative_bias_table[relative_index]  # [N, N, H]
    in_maps = []
    for h in range(HEADS):
        w96 = np.concatenate(
            [w_qkv[:, h * D:(h + 1) * D],
             w_qkv[:, 256 + h * D:256 + (h + 1) * D],
             w_qkv[:, 512 + h * D:512 + (h + 1) * D]], axis=1)  # [384, 96]
        wqkv3 = np.ascontiguousarray(w96.reshape(3, 128, 96)).astype(bf)
        wout4 = np.tile(w_out[h * D:(h + 1) * D, :], (4, 1)).astype(np.float32)
        expbT = np.exp(bias_full[:, :, h].T)  # [j, i]
        expb = np.ascontiguousarray(
            expbT.reshape(NJC, 128, N).transpose(1, 0, 2)).astype(bf)
        in_maps.append({
            "xt": xt, "wqkv": wqkv3, "wout4": wout4.astype(bf),
            "expb": expb, "ident": ident,
            "identb": np.eye(128, dtype=np.float32).astype(bf),
        })
    return in_maps


_NC_CACHE = {}


def kernel(x, w_qkv, relative_bias_table, w_out, b_out, relative_index):
    x = np.asarray(x, dtype=np.float32)
    w_qkv = np.asarray(w_qkv, dtype=np.float32)
    relative_bias_table = np.asarray(relative_bias_table, dtype=np.float32)
    w_out = np.asarray(w_out, dtype=np.float32)
    b_out = np.asarray(b_out, dtype=np.float32)
    relative_index = np.asarray(relative_index)

    NB, N, _ = x.shape
    key = (NB, N)
    if key not in _NC_CACHE:
        _NC_CACHE[key] = build_kernel(NB=NB, N=N, num_devices=HEADS)
    nc = _NC_CACHE[key]

    in_maps = host_prep(x, w_qkv, relative_bias_table, relative_index, w_out, NB, N)
    res = run_bass_kernel_spmd(nc, in_maps, core_ids=list(range(HEADS)))
    out = np.zeros((NB * N, OUP), np.float32)
    for r in res.results:
        out += r["outp"].astype(np.float32)
    out += b_out[None, :]
    return out.reshape(NB, N, OUP)
